# revision 56
# baseline (speedup 1.0000x reference)
"""Trainium2 Bass kernel for a 2-layer heterogeneous GNN (HGNN).

Graph: users/items (50000 each), 3 relations (follows: u->u, rates: u->i,
ratedby: i->u), 800000 edges per relation, GraphConv with norm='both',
HeteroGraphConv mean aggregation over relations per destination type.

Strategy v2 (8 NeuronCores, SPMD single program):
  - Destination-node sharding: core c owns destination rows
    [c*6250, (c+1)*6250) of both node tables for every relation.
  - W is commuted past the segment-sum (linearity):
        segsum(rs_o[s]*rs_i[d]*x[s]) @ W
    so layer 1 gathers RAW x rows straight from host-staged replicated
    tables (no feature matmul pass, no first AllGather), and both
    normalizations (and the 0.5 relation-mean) are folded into the
    one-hot S matrix values, precomputed per edge on the host.
  - SpMM: per 128-edge tile, S[e, d] = sval[e] * (iota[d] == dstrel[e])
    is built with ONE 4x-mode DVE tensor_scalar, then
      layer 1 (G-stationary): psum[f_half, d] += G[:, f_half]^T-mm S
        -> psum is directly A~^T, consumed as lhsT by the per-window
           transform matmuls A~ @ W1 (no transposes),
      layer 2 (S-stationary): psum[d, j] += S^T-mm G2 (output layout).
  - Gathers are issued as few, large dma_gather calls (multi-window
    groups, static per-window slot budgets, interior pad slots point at
    row 0 with sval 0): SWDGE descriptor-generation cost ~994ns/call is
    amortized; no count registers, no memsets.
  - flush1 per dst window: h = relu(psum + b1), PE-transpose of h for
    f2_r = h @ W2_r feature rows (bf16), written to per-chunk DRAM
    tensors that are AllGather'ed in chunks overlapped with layer-1
    compute.  Layer 2 gathers f2 rows and flushes fp32 outputs.
"""

import os

import numpy as np
import ml_dtypes

import concourse.bacc as bacc
import concourse.bass as bass
import concourse.tile as tile
from concourse import mybir
from concourse.bass_utils import run_bass_kernel_spmd

BF16 = mybir.dt.bfloat16
F32 = mybir.dt.float32
I16 = mybir.dt.int16

NCORE = 8
N = 50000
E = 800000
D_IN = 256
D_HID = 256
D_OUT = 128
SLAB = N // NCORE          # 6250 destination rows per core
WPC = (SLAB + 127) // 128  # 49 windows of 128 dst rows
SLAB_PAD = WPC * 128       # 6272
PADN = NCORE * SLAB_PAD    # 50176 rows in padded tables
CHUNK0 = 32768             # int16 index limit: src chunk boundary

RELS = ("follows", "rates", "ratedby")
SRC_IS_USER = {"follows": True, "rates": True, "ratedby": False}
HALF = {"follows": 0.5, "rates": 1.0, "ratedby": 0.5}

NW1 = 2   # dst windows per gather group, layer 1 (512B elems)
NW2 = 5   # layer 2 (256B elems)
AGW = int(os.environ.get("AGW", "8"))  # dst windows per AllGather chunk

_CACHE = {}
LAST_RESULT = None
DBG_PHASES = os.environ.get("DBG_PHASES", "BC")  # B = layer 1, C = layer 2
# SWDGE descriptor ring fits ~128 descs/engine per queue: 1024 idx per call
# (64+1 descs/engine) is proven safe; larger calls deadlock the ring.
MAXIDX = int(os.environ.get("MAXIDX", "1024"))  # gather sub-call cap
NQ = int(os.environ.get("NQ", "4"))  # SWDGE queues


def _r128(x):
    return ((np.asarray(x) + 127) // 128) * 128


def _pack_idx(lin):
    """[8, L] int16 -> [8, 128, L//16] wrapped (slot s at [s%16, s//16]),
    replicated across the 8 Q7-core partition groups."""
    L = lin.shape[1]
    a = lin.reshape(NCORE, L // 16, 16).transpose(0, 2, 1)
    return np.ascontiguousarray(np.tile(a, (1, 8, 1)))


def _pack128(lin):
    """[8, L] f32 -> [8, 128, L//128] bf16 (slot s at [s%128, s//128])."""
    L = lin.shape[1]
    return np.ascontiguousarray(
        lin.reshape(NCORE, L // 128, 128).transpose(0, 2, 1)
    ).astype(ml_dtypes.bfloat16)


def _groups(nw):
    gs = []
    w0 = 0
    while w0 < WPC:
        gs.append((w0, min(w0 + nw, WPC)))
        w0 += nw
    return gs


def _agchunks():
    cs = []
    w0 = 0
    while w0 < WPC:
        cs.append((w0, min(w0 + AGW, WPC)))
        w0 += AGW
    return cs


def _l2row_map():
    """Map padded row (core, w, rr) -> chunk-major f2f row.

    f2f layout: AG chunk regions in order; region ci holds
    [rank0 windows w0..w1) | rank1 ... ] contiguously."""
    rowmap = np.empty(PADN, np.int64)
    reg_off = 0
    for (w0, w1) in _agchunks():
        nw = w1 - w0
        for c in range(NCORE):
            for w in range(w0, w1):
                src_base = c * SLAB_PAD + w * 128
                dst_base = reg_off + (c * nw + (w - w0)) * 128
                rowmap[src_base : src_base + 128] = np.arange(
                    dst_base, dst_base + 128
                )
        reg_off += NCORE * nw * 128
    assert reg_off == PADN
    return rowmap


def _layout(rowidx, core, w, drel):
    """Lay out one relation's edges into per-window static slot budgets
    (multiples of 128, uniform across cores), split at CHUNK0 by row
    index, sorted by row within a segment.  Pad slots: idx 0, dstrel
    -1."""
    chunk = (rowidx >= CHUNK0).astype(np.int64)
    key = (core * 2 + chunk) * WPC + w
    order = np.lexsort((rowidx, key))
    key_s = key[order]
    counts = np.bincount(key, minlength=NCORE * 2 * WPC)
    starts = np.concatenate(([0], np.cumsum(counts)[:-1]))
    pos = np.arange(E, dtype=np.int64) - starts[key_s]

    cnt = counts.reshape(NCORE, 2, WPC)
    TA = np.maximum(_r128(cnt[:, 0, :].max(axis=0)), 128)  # [WPC]
    TB = np.maximum(_r128(cnt[:, 1, :].max(axis=0)), 128)
    offA = np.concatenate(([0], np.cumsum(TA))).astype(np.int64)
    offB = np.concatenate(([0], np.cumsum(TB))).astype(np.int64)
    LA, LB = int(offA[-1]), int(offB[-1])

    idxA = np.zeros((NCORE, LA), np.int16)
    dstA = np.full((NCORE, LA), -1.0, np.float32)
    idxB = np.zeros((NCORE, LB), np.int16)
    dstB = np.full((NCORE, LB), -1.0, np.float32)

    core_s = core[order]
    ch_s = chunk[order]
    w_s = w[order]
    sp_s = rowidx[order]
    dr_s = drel[order]

    mA = ch_s == 0
    slotA = offA[w_s[mA]] + pos[mA]
    idxA[core_s[mA], slotA] = sp_s[mA].astype(np.int16)
    dstA[core_s[mA], slotA] = dr_s[mA]
    mB = ~mA
    slotB = offB[w_s[mB]] + pos[mB]
    idxB[core_s[mB], slotB] = (sp_s[mB] - CHUNK0).astype(np.int16)
    dstB[core_s[mB], slotB] = dr_s[mB]

    return {
        "TA": TA, "TB": TB, "offA": offA, "offB": offB, "LA": LA, "LB": LB,
        "idxA": _pack_idx(idxA), "idxB": _pack_idx(idxB),
        "dstA": _pack128(dstA), "dstB": _pack128(dstB),
    }


def _prep_relation(src, dst, l2map):
    """Per-edge metadata + layer-1 (padded-table) and layer-2
    (chunk-major f2f) slot layouts for one relation."""
    src = np.asarray(src).astype(np.int64)
    dst = np.asarray(dst).astype(np.int64)

    core = dst // SLAB
    dst_loc = dst - core * SLAB
    w = dst_loc >> 7
    drel = (dst_loc & 127).astype(np.float32)
    src_pad = src + (SLAB_PAD - SLAB) * (src // SLAB)

    return {
        "1": _layout(src_pad, core, w, drel),
        "2": _layout(l2map[src_pad], core, w, drel),
    }


def _build_streams(meta, nw, lk):
    """Pack per-group metadata DMA blocks: for each group of nw windows,
    one int16 block per rel: [ixa | ixb | da | db] (da/db are bf16
    dstrel columns viewed as int16).  lk: "1" or "2" (which layout).

    Returns (arr [8, 128, total_cols], layout).  Layout entry:
    (col0, cols, per-rel offset dict); offsets relative to block start.
    """
    blocks = []
    layout = []
    for (w0, w1) in _groups(nw):
        parts = []
        rl = {}
        ci = 0
        for r in RELS:
            m = meta[r][lk]
            a0, a1 = int(m["offA"][w0]), int(m["offA"][w1])
            b0, b1 = int(m["offB"][w0]), int(m["offB"][w1])
            na, nb = a1 - a0, b1 - b0
            seg = {"na": na, "nb": nb}
            seg["ixa"] = ci
            parts.append(m["idxA"][:, :, a0 // 16 : a1 // 16])
            ci += na // 16
            seg["ixb"] = ci
            parts.append(m["idxB"][:, :, b0 // 16 : b1 // 16])
            ci += nb // 16
            for nm, arr, o0, o1 in (
                ("da", m["dstA"], a0, a1), ("db", m["dstB"], b0, b1),
            ):
                n = (o1 - o0) // 128
                seg[nm] = ci
                parts.append(
                    arr[:, :, o0 // 128 : o1 // 128].view(np.int16)
                )
                ci += n
            rl[r] = seg
        blocks.append(np.concatenate(parts, axis=2))
        layout.append((rl, ci))
    arr = np.concatenate(blocks, axis=2)
    c0 = 0
    lay = []
    for (rl, ci) in layout:
        lay.append((c0, ci, rl))
        c0 += ci
    return np.ascontiguousarray(arr), lay


# ------------------------------------------------------------ device program


def _build(meta):
    nc = bacc.Bacc(
        "TRN2", debug=False, dynamic_dma_scratch_size=32768,
        num_swdge_queues=NQ,
    )

    inp = {}

    def din(name, shape, dt):
        inp[name] = nc.dram_tensor(name, list(shape), dt, kind="ExternalInput")
        return inp[name]

    for r in RELS:
        din(f"xn_{r}", (PADN, D_IN), BF16)  # src table scaled by rs_out
        din(f"w1_{r}", (128, 2, D_HID), BF16)
        din(f"w2_{r}", (128, 2, D_OUT), BF16)
        din(f"rso_{r}", (128, WPC), F32)  # rs_out slabs (f2 write scale)
    din("iota", (128, 128), BF16)
    din("ident", (128, 128), BF16)
    din("rsif", (128, WPC), F32)   # 0.5 * rs_in follows  (user rows)
    din("rsirb", (128, WPC), F32)  # 0.5 * rs_in ratedby  (user rows)
    din("rsii", (128, WPC), F32)   # rs_in rates          (item rows)
    din("b1u", (128, D_HID), F32)
    din("b1i", (128, D_HID), F32)
    din("b2u", (128, D_OUT), F32)
    din("b2i", (128, D_OUT), F32)
    din("st1", (128, meta["lay1_ci"]), I16)
    din("st2", (128, meta["lay2_ci"]), I16)

    ou = nc.dram_tensor("ou", [SLAB_PAD, D_OUT], F32, kind="ExternalOutput")
    oi = nc.dram_tensor("oi", [SLAB_PAD, D_OUT], F32, kind="ExternalOutput")

    # f2 per AllGather chunk (so chunked AGs only depend on their windows)
    agchunks = _agchunks()
    regoff = [0]
    for (c0, c1) in agchunks:
        regoff.append(regoff[-1] + NCORE * (c1 - c0) * 128)
    f2c = {
        r: [
            nc.dram_tensor(f"f2_{r}_{ci}", [(c1 - c0) * 128, D_OUT], BF16)
            for ci, (c0, c1) in enumerate(agchunks)
        ]
        for r in RELS
    }
    f2f = {
        r: nc.dram_tensor(f"f2f_{r}", [PADN, D_OUT], BF16, addr_space="Shared")
        for r in RELS
    }

    eq = mybir.AluOpType.is_equal
    mult = mybir.AluOpType.mult
    add = mybir.AluOpType.add
    rg = [list(range(NCORE))]
    relu = mybir.ActivationFunctionType.Relu
    actcopy = mybir.ActivationFunctionType.Copy

    lay1 = meta["lay1"]
    lay2 = meta["lay2"]
    qctr = [0]

    phase_b = "B" in DBG_PHASES
    phase_c = "C" in DBG_PHASES

    with tile.TileContext(nc) as tc:
        with tc.tile_pool(name="const", bufs=1) as cpool:
            w1_sb = {}
            w2_sb = {}
            rso_sb = {}
            for r in RELS:
                w1_sb[r] = cpool.tile([128, 2, D_HID], BF16, tag=f"w1{r}", name=f"w1_{r}")
                nc.sync.dma_start(w1_sb[r][:], inp[f"w1_{r}"][:])
                w2_sb[r] = cpool.tile([128, 2, D_OUT], BF16, tag=f"w2{r}", name=f"w2_{r}")
                nc.sync.dma_start(w2_sb[r][:], inp[f"w2_{r}"][:])
                rso_sb[r] = cpool.tile([128, WPC], F32, tag=f"rso{r}", name=f"rso_{r}")
                nc.sync.dma_start(rso_sb[r][:], inp[f"rso_{r}"][:])
            iota_sb = cpool.tile([128, 128], BF16, tag="iota")
            nc.sync.dma_start(iota_sb[:], inp["iota"][:])
            ident_sb = cpool.tile([128, 128], BF16, tag="ident")
            nc.sync.dma_start(ident_sb[:], inp["ident"][:])
            rsif = cpool.tile([128, WPC], F32, tag="rsif")
            nc.sync.dma_start(rsif[:], inp["rsif"][:])
            rsirb = cpool.tile([128, WPC], F32, tag="rsirb")
            nc.sync.dma_start(rsirb[:], inp["rsirb"][:])
            rsii = cpool.tile([128, WPC], F32, tag="rsii")
            nc.sync.dma_start(rsii[:], inp["rsii"][:])
            b1u = cpool.tile([128, D_HID], F32, tag="b1u")
            nc.sync.dma_start(b1u[:], inp["b1u"][:])
            b1i = cpool.tile([128, D_HID], F32, tag="b1i")
            nc.sync.dma_start(b1i[:], inp["b1i"][:])
            b2u = cpool.tile([128, D_OUT], F32, tag="b2u")
            nc.sync.dma_start(b2u[:], inp["b2u"][:])
            b2i = cpool.tile([128, D_OUT], F32, tag="b2i")
            nc.sync.dma_start(b2i[:], inp["b2i"][:])

            def seg_pass(layer, lay, stin, gsrc, d, flush):
                """One gather+segment pass over all windows.

                lay: per-group (c0, cols, rl) layout of the packed
                metadata stream tensor.  gsrc[r]: DRAM table AP,
                d: feature dim.  flush(w, psums) consumes per-relation
                PSUM tiles for window w."""
                maxci = max(e[1] for e in lay)
                maxta = {
                    r: max(e[2][r]["na"] // 128 for e in lay) for r in RELS
                }
                maxtb = {
                    r: max(e[2][r]["nb"] // 128 for e in lay) for r in RELS
                }
                maxnt = 0
                for r in RELS:
                    m = meta[r][str(layer)]
                    maxnt = max(
                        maxnt, int(m["TA"].max()) // 128,
                        int(m["TB"].max()) // 128,
                    )
                gidx = 0
                for (ci0, ncI, rl) in lay:
                    st = gpool.tile([128, maxci], I16, tag=f"st{layer}")
                    nc.sync.dma_start(
                        st[:, :ncI], stin[:, ci0 : ci0 + ncI]
                    )
                    gt = {}
                    for r in RELS:
                        seg = rl[r]
                        na, nb = seg["na"], seg["nb"]
                        ga = gpool.tile(
                            [128, maxta[r], d], BF16, tag=f"ga{layer}{r}"
                        )
                        gb = gpool.tile(
                            [128, maxtb[r], d], BF16, tag=f"gb{layer}{r}"
                        )
                        for (gbuf, n, src_ap, ix0) in (
                            (ga, na, gsrc[r][0:CHUNK0, :], seg["ixa"]),
                            (gb, nb, gsrc[r][CHUNK0:PADN, :], seg["ixb"]),
                        ):
                            for k0 in range(0, n, MAXIDX):
                                k1 = min(k0 + MAXIDX, n)
                                nc.gpsimd.dma_gather(
                                    gbuf[:, k0 // 128 : k1 // 128, :],
                                    src_ap,
                                    st[:, ix0 + k0 // 16 : ix0 + k1 // 16],
                                    k1 - k0,
                                    k1 - k0,
                                    d,
                                    single_packet=True,
                                    queue_num=qctr[0] % NQ,
                                )
                                qctr[0] += 1
                        gt[r] = (ga, gb)
                    # process windows of this group
                    w0 = meta["groups"][layer][gidx][0]
                    w1 = meta["groups"][layer][gidx][1]
                    gidx += 1
                    for w in range(w0, w1):
                        psums = {}
                        for ri, r in enumerate(RELS):
                            seg = rl[r]
                            ga, gb = gt[r]
                            m = meta[r][str(layer)]
                            # tile offsets of window w inside this group
                            ta0 = int((m["offA"][w] - m["offA"][w0]) // 128)
                            ta1 = int((m["offA"][w + 1] - m["offA"][w0]) // 128)
                            tb0 = int((m["offB"][w] - m["offB"][w0]) // 128)
                            tb1 = int((m["offB"][w + 1] - m["offB"][w0]) // 128)
                            nt = (ta1 - ta0) + (tb1 - tb0)
                            ps = pspool.tile(
                                [128, d], F32, tag=f"ps{layer}{r}",
                                name=f"ps{layer}{r}",
                            )
                            psums[r] = ps
                            k = 0
                            for (gbuf, t0, t1, cname) in (
                                (ga, ta0, ta1, "da"),
                                (gb, tb0, tb1, "db"),
                            ):
                                nseg = t1 - t0
                                if nseg == 0:
                                    continue
                                cbase = seg[cname]
                                stile = spool.tile(
                                    [128, maxnt, 128], BF16, tag=f"S{layer}"
                                )
                                nc.vector.tensor_tensor(
                                    stile[:, :nseg, :],
                                    iota_sb[:, :]
                                    .rearrange("p (o f) -> p o f", o=1)
                                    .broadcast_to([128, nseg, 128]),
                                    st[:, cbase + t0 : cbase + t1]
                                    .bitcast(BF16)
                                    .rearrange("p (t o) -> p t o", o=1)
                                    .broadcast_to([128, nseg, 128]),
                                    eq,
                                )
                                for t in range(t0, t1):
                                    nc.tensor.matmul(
                                        ps[:],
                                        stile[:, t - t0, :],
                                        gbuf[:, t, :],
                                        start=(k == 0),
                                        stop=(k == nt - 1),
                                    )
                                    k += 1
                        flush(w, psums)

            # ---------------- phase B: layer-1 SpMM + h + f2 features
            with (
                tc.tile_pool(name="g1", bufs=2) as gpool,
                tc.tile_pool(name="ps1", bufs=1, space="PSUM") as pspool,
                tc.tile_pool(name="s1", bufs=4) as spool,
                tc.tile_pool(name="fl1", bufs=2) as flpool,
                tc.tile_pool(name="psh", bufs=2, space="PSUM") as pshpool,
                tc.tile_pool(name="pst", bufs=1, space="PSUM") as pstpool,
                tc.tile_pool(name="ps2f", bufs=2, space="PSUM") as ps2fpool,
            ):

                def flush1(w, psums):
                    # A~ -> bf16, transpose to get lhsT, transform @ W1
                    phs = {}
                    for ri, r in enumerate(RELS):
                        ab = flpool.tile([128, D_HID], BF16, tag=f"ab{r}",
                                         name=f"ab{r}")
                        nc.scalar.activation(ab[:], psums[r][:], actcopy)
                        atT = flpool.tile([128, 2, 128], BF16, tag=f"atT{r}",
                                          name=f"atT{r}")
                        for h in range(2):
                            pt = pstpool.tile([128, 128], BF16, tag="pst")
                            nc.tensor.transpose(
                                pt[:], ab[:, h * 128 : (h + 1) * 128],
                                ident_sb[:],
                            )
                            nc.vector.tensor_copy(atT[:, h, :], pt[:])
                        ph = pshpool.tile([128, D_HID], F32, tag="ph",
                                          name=f"ph{r}")
                        for h in range(2):
                            nc.tensor.matmul(
                                ph[:],
                                atT[:, h, :],
                                w1_sb[r][:, h, :],
                                start=(h == 0),
                                stop=(h == 1),
                            )
                        phs[r] = ph
                    hts = {}
                    for nm, specs, btile in (
                        ("u", (("follows", rsif), ("ratedby", rsirb)), b1u),
                        ("i", (("rates", rsii),), b1i),
                    ):
                        t = flpool.tile([128, D_HID], F32, tag=f"hb{nm}",
                                        name=f"hb{nm}")
                        (r0, rs0) = specs[0]
                        nc.vector.scalar_tensor_tensor(
                            t[:], phs[r0][:], rs0[:, w : w + 1], btile[:],
                            mult, add,
                        )
                        if len(specs) > 1:
                            (r1, rs1) = specs[1]
                            t2 = flpool.tile([128, D_HID], F32,
                                             tag=f"hc{nm}", name=f"hc{nm}")
                            nc.vector.scalar_tensor_tensor(
                                t2[:], phs[r1][:], rs1[:, w : w + 1],
                                t[:], mult, add,
                            )
                            t = t2
                        hr = flpool.tile([128, D_HID], BF16, tag=f"h{nm}",
                                         name=f"h{nm}")
                        nc.scalar.activation(hr[:], t[:], relu)
                        # transpose h for the f2 matmuls
                        hT = flpool.tile([128, 2, 128], BF16, tag=f"hT{nm}",
                                         name=f"hT{nm}")
                        for h in range(2):
                            pt = pstpool.tile([128, 128], BF16, tag="pst")
                            nc.tensor.transpose(
                                pt[:], hr[:, h * 128 : (h + 1) * 128],
                                ident_sb[:],
                            )
                            nc.vector.tensor_copy(hT[:, h, :], pt[:])
                        hts[nm] = hT
                    ci, cw0 = meta["agw"][w]
                    for r in RELS:
                        hT = hts["u"] if SRC_IS_USER[r] else hts["i"]
                        p2 = ps2fpool.tile([128, D_OUT], F32, tag="p2f")
                        for h in range(2):
                            nc.tensor.matmul(
                                p2[:], hT[:, h, :], w2_sb[r][:, h, :],
                                start=(h == 0), stop=(h == 1),
                            )
                        # scale rows by rs_out (f2~ = rs_o * h @ W2)
                        f2t = flpool.tile([128, D_OUT], BF16, tag="f2t")
                        nc.scalar.activation(
                            f2t[:], p2[:], actcopy,
                            scale=rso_sb[r][:, w : w + 1],
                        )
                        nc.sync.dma_start(
                            f2c[r][ci][
                                (w - cw0) * 128 : (w - cw0 + 1) * 128, :
                            ],
                            f2t[:],
                        )
                    # chunked AllGather once the chunk's last window is done
                    c0, c1 = agchunks[ci]
                    if w == c1 - 1 and phase_c:
                        for r in RELS:
                            out_ap = f2f[r][regoff[ci] : regoff[ci + 1], :]
                            nc.gpsimd.collective_compute(
                                "AllGather",
                                mybir.AluOpType.bypass,
                                replica_groups=rg,
                                ins=[f2c[r][ci].ap().opt()],
                                outs=[out_ap.opt()],
                            )

                if phase_b:
                    seg_pass(
                        1, lay1, inp["st1"],
                        {r: inp[f"xn_{r}"] for r in RELS},
                        D_IN, flush1,
                    )

            # ---------------- phase C: layer-2 SpMM -> outputs
            with (
                tc.tile_pool(name="g2", bufs=2) as gpool,
                tc.tile_pool(name="ps2", bufs=2, space="PSUM") as pspool,
                tc.tile_pool(name="s2", bufs=4) as spool,
                tc.tile_pool(name="fl2", bufs=3) as flpool,
            ):

                def flush2(w, psums):
                    t1 = flpool.tile([128, D_OUT], F32, tag="o1")
                    nc.vector.scalar_tensor_tensor(
                        t1[:], psums["follows"][:], rsif[:, w : w + 1],
                        b2u[:], mult, add,
                    )
                    out_u = flpool.tile([128, D_OUT], F32, tag="ou")
                    nc.vector.scalar_tensor_tensor(
                        out_u[:], psums["ratedby"][:], rsirb[:, w : w + 1],
                        t1[:], mult, add,
                    )
                    nc.sync.dma_start(ou[w * 128 : (w + 1) * 128, :], out_u[:])
                    out_i = flpool.tile([128, D_OUT], F32, tag="oiT")
                    nc.vector.scalar_tensor_tensor(
                        out_i[:], psums["rates"][:], rsii[:, w : w + 1],
                        b2i[:], mult, add,
                    )
                    nc.sync.dma_start(oi[w * 128 : (w + 1) * 128, :], out_i[:])

                if phase_c:
                    seg_pass(
                        2, lay2, inp["st2"],
                        {r: f2f[r] for r in RELS}, D_OUT, flush2,
                    )

    nc.compile()
    return nc


# ------------------------------------------------------------------- kernel


def prepare(inputs):
    """Host-side prep: returns (meta, in_maps)."""
    bf = ml_dtypes.bfloat16
    l2map = _l2row_map()
    meta = {}
    for r in RELS:
        meta[r] = _prep_relation(
            inputs[f"src_{r}"], inputs[f"dst_{r}"], l2map
        )

    st1, lay1 = _build_streams(meta, NW1, "1")
    st2, lay2 = _build_streams(meta, NW2, "2")
    meta["lay1"] = lay1
    meta["lay2"] = lay2
    meta["lay1_ci"] = st1.shape[2]
    meta["lay2_ci"] = st2.shape[2]
    meta["groups"] = {1: _groups(NW1), 2: _groups(NW2)}
    agw = {}
    ci = 0
    w0 = 0
    while w0 < WPC:
        w1 = min(w0 + AGW, WPC)
        for w in range(w0, w1):
            agw[w] = (ci, w0)
        ci += 1
        w0 = w1
    meta["agw"] = agw

    def padtab(x, scale):
        t = np.zeros((PADN, D_IN), bf)
        xv = (np.asarray(x, np.float32) * scale[:, None]).reshape(
            NCORE, SLAB, D_IN
        )
        tv = t.reshape(NCORE, SLAB_PAD, D_IN)
        tv[:, :SLAB, :] = xv.astype(bf)
        return np.ascontiguousarray(t)

    def rs_slabs(v):
        """[N] f32 -> per-core [8, 128, WPC] slab columns."""
        vp = np.ones(PADN, np.float32)
        vp_view = vp.reshape(NCORE, SLAB_PAD)
        vp_view[:, :SLAB] = v.reshape(NCORE, SLAB)
        return np.ascontiguousarray(
            vp_view.reshape(NCORE, WPC, 128).transpose(0, 2, 1)
        )

    rs_o = {}
    rs_i = {}
    for r in RELS:
        src = np.asarray(inputs[f"src_{r}"]).astype(np.int64)
        dst = np.asarray(inputs[f"dst_{r}"]).astype(np.int64)
        rs_o[r] = (
            1.0 / np.sqrt(np.maximum(np.bincount(src, minlength=N), 1.0))
        ).astype(np.float32)
        rs_i[r] = (
            1.0 / np.sqrt(np.maximum(np.bincount(dst, minlength=N), 1.0))
        ).astype(np.float32)

    xn = {}
    rso = {}
    for r in RELS:
        x = inputs["x_user"] if SRC_IS_USER[r] else inputs["x_item"]
        xn[r] = padtab(x, rs_o[r])
        rso[r] = rs_slabs(rs_o[r])

    rsif = rs_slabs(0.5 * rs_i["follows"])
    rsirb = rs_slabs(0.5 * rs_i["ratedby"])
    rsii = rs_slabs(rs_i["rates"])

    iota = np.broadcast_to(
        np.arange(128, dtype=np.float32), (128, 128)
    ).astype(bf)
    ident = np.eye(128, dtype=np.float32).astype(bf)

    b1u = np.broadcast_to(
        0.5 * (np.asarray(inputs["b1_follows"], np.float32)
               + np.asarray(inputs["b1_ratedby"], np.float32)),
        (128, D_HID),
    ).astype(np.float32)
    b1i = np.broadcast_to(
        np.asarray(inputs["b1_rates"], np.float32), (128, D_HID)
    ).astype(np.float32)
    b2u = np.broadcast_to(
        0.5 * (np.asarray(inputs["b2_follows"], np.float32)
               + np.asarray(inputs["b2_ratedby"], np.float32)),
        (128, D_OUT),
    ).astype(np.float32)
    b2i = np.broadcast_to(
        np.asarray(inputs["b2_rates"], np.float32), (128, D_OUT)
    ).astype(np.float32)

    w1 = {
        r: np.ascontiguousarray(
            np.asarray(inputs[f"W1_{r}"], np.float32)
            .astype(bf).reshape(2, 128, D_HID).transpose(1, 0, 2)
        )
        for r in RELS
    }
    w2 = {
        r: np.ascontiguousarray(
            np.asarray(inputs[f"W2_{r}"], np.float32)
            .astype(bf).reshape(2, 128, D_OUT).transpose(1, 0, 2)
        )
        for r in RELS
    }

    in_maps = []
    for c in range(NCORE):
        m = {
            "iota": iota, "ident": ident,
            "rsif": rsif[c], "rsirb": rsirb[c], "rsii": rsii[c],
            "b1u": b1u, "b1i": b1i, "b2u": b2u, "b2i": b2i,
            "st1": np.ascontiguousarray(st1[c]),
            "st2": np.ascontiguousarray(st2[c]),
        }
        for r in RELS:
            m[f"xn_{r}"] = xn[r]
            m[f"w1_{r}"] = w1[r]
            m[f"w2_{r}"] = w2[r]
            m[f"rso_{r}"] = rso[r][c]
        in_maps.append(m)
    return meta, in_maps


def kernel(**inputs):
    key = tuple(
        (int(np.asarray(inputs[f"src_{r}"][:97]).sum()),
         int(np.asarray(inputs[f"dst_{r}"][:97]).sum()))
        for r in RELS
    )
    meta, in_maps = prepare(inputs)
    if key not in _CACHE:
        _CACHE[key] = _build(meta)
    nc = _CACHE[key]

    global LAST_RESULT
    res = run_bass_kernel_spmd(nc, in_maps, list(range(NCORE)))
    LAST_RESULT = res

    o_u = np.concatenate(
        [res.results[c]["ou"][:SLAB] for c in range(NCORE)], axis=0
    )
    o_i = np.concatenate(
        [res.results[c]["oi"][:SLAB] for c in range(NCORE)], axis=0
    )
    return (o_u, o_i)
